# revision 7
# baseline (speedup 1.0000x reference)
"""Trainium2 Bass kernel for nn_Dial2vec (dialogue contrastive pretraining loss).

Strategy
--------
All three role-masked tensors are per-token scalar multiples of the same
hidden states: q_so[t] = cq[t]*x[t], etc.  Hence the role-masked cross scores
collapse onto ONE Gram matrix G = x @ x.T via a rank-4 outer-product mask
    K = sg sg^T - cq cq^T - ca ca^T - cn cn^T          (sg = cq+ca+cn)
and the cross outputs are only ever consumed through masked averages, which
turn the [S,S]@[S,H] matmuls into weighted vector-matrix products.

Per sample the device computes (fp16 operands, fp32 PSUM accumulation):
    G   = x16 x16^T   (upper-triangular blocks only; G is symmetric)
    Gb  = G * band    (band from sorted turn_ids: |iota - c_s| <= w_s,
                       fused DVE op; lower blocks = PE-transposed upper)
    P12 = V12^T Gb ;  P3^T = p12b^T E3 ;  u3^T = P3^T * C3^T
    numT[h,j] = x^T [selfw | u3^T]      (fp32, N=6 matmuls)

Everything runs in fp16 with fp32 accumulation (measured: loss rel-err
~2e-5, output tensors ~2e-4); the cross path carries a 1/16 scale for fp16
dynamic-range headroom.  Host does the cheap O(B*S) mask precompute and
the final divisions / cosine / log-softmax / nanmean on [8,9] tensors.

Sharding: data-parallel over samples, 72 = 8 cores x 9 samples.
"""

import numpy as np
import ml_dtypes

S = 512
H = 768
B = 72
NCORES = 8
PER = B // NCORES          # 9 samples per core
NCH = S // 128             # 4 partition chunks of the sequence
NHB = H // 128             # 6 hidden blocks
SAMPLE_NUMS = 9
VIEW_RANGE = 2
TEMP = 0.07
AVG_EPS = 1e-6
COS_EPS = 1e-8

_NC_CACHE = None


def _build_nc():
    import concourse.bacc as bacc
    import concourse.bass as bass
    import concourse.tile as tile
    import concourse.mybir as mybir
    from concourse.masks import make_identity

    fp32 = mybir.dt.float32
    fp16 = mybir.dt.float16
    Alu = mybir.AluOpType
    Act = mybir.ActivationFunctionType

    nc = bacc.Bacc("TRN2", target_bir_lowering=False, debug=False)
    x_d = nc.dram_tensor("x16", [PER, S, H], fp16, kind="ExternalInput").ap()
    v12_d = nc.dram_tensor("v12", [PER, S, 12], fp16, kind="ExternalInput").ap()
    b12_d = nc.dram_tensor("b12", [PER, 12, S], fp32, kind="ExternalInput").ap()
    c3t_d = nc.dram_tensor("c3t", [PER, S, 3], fp32, kind="ExternalInput").ap()
    selfw_d = nc.dram_tensor("selfw", [PER, S, 3], fp16, kind="ExternalInput").ap()
    negc_d = nc.dram_tensor("negc", [PER, S], fp32, kind="ExternalInput").ap()
    wband_d = nc.dram_tensor("wband", [PER, S], fp32, kind="ExternalInput").ap()
    iota_d = nc.dram_tensor("iota", [1, S], fp32, kind="ExternalInput").ap()
    e3_d = nc.dram_tensor("e3", [12, 3], fp16, kind="ExternalInput").ap()
    out_d = nc.dram_tensor("out", [PER, 6, H], fp32, kind="ExternalOutput").ap()

    with tile.TileContext(nc) as tc:
        with (
            tc.tile_pool(name="const", bufs=1) as constp,
            tc.tile_pool(name="xt", bufs=3) as xtp,
            tc.tile_pool(name="xh", bufs=3) as xhp,
            tc.tile_pool(name="gb", bufs=8) as gbp,
            tc.tile_pool(name="work", bufs=3) as workp,
            tc.tile_pool(name="aux", bufs=3) as auxp,
            tc.tile_pool(name="psum", bufs=1, space=bass.MemorySpace.PSUM) as psp,
        ):
            eye_h = constp.tile([128, 128], fp16, name="eye_h")
            make_identity(nc, eye_h[:, :])
            iota_sb = constp.tile([128, S], fp32, name="iota_sb")
            nc.sync.dma_start(
                out=iota_sb[:, :],
                in_=bass.AP(tensor=iota_d.tensor, offset=0, ap=[[0, 128], [1, S]]),
            )
            e3_sb = constp.tile([12, 3], fp16, name="e3_sb")
            nc.sync.dma_start(out=e3_sb[:, :], in_=e3_d)

            for i in range(PER):
                # ---- loads: x on the sync ring, transposes on the scalar
                # ring, aux via gpsimd SWDGE ----
                xt = xtp.tile([128, NCH, H], fp16, tag="xt", bufs=3)
                xh = xhp.tile([128, NHB, S], fp16, tag="xh", bufs=3)
                xi = x_d[i].rearrange("(c p) h -> p c h", p=128)
                for c in range(NCH):
                    nc.sync.dma_start(out=xt[:, c, :], in_=xi[:, c, :])
                    nc.scalar.dma_start_transpose(
                        out=xh[:, :, c * 128 : (c + 1) * 128], in_=xt[:, c, :]
                    )
                v12_sb = auxp.tile([128, NCH, 12], fp16, tag="v12", bufs=3)
                nc.gpsimd.dma_start(
                    out=v12_sb[:, :, :],
                    in_=v12_d[i].rearrange("(c p) r -> p c r", p=128),
                )
                b12_sb = auxp.tile([12, S], fp32, tag="b12", bufs=3)
                nc.gpsimd.dma_start(out=b12_sb[:, :], in_=b12_d[i])
                c3t_sb = auxp.tile([128, NCH, 3], fp32, tag="c3t", bufs=3)
                nc.gpsimd.dma_start(
                    out=c3t_sb[:, :, :],
                    in_=c3t_d[i].rearrange("(c p) r -> p c r", p=128),
                )
                negc_sb = auxp.tile([128, NCH], fp32, tag="negc", bufs=3)
                nc.gpsimd.dma_start(
                    out=negc_sb[:, :], in_=negc_d[i].rearrange("(c p) -> p c", p=128)
                )
                wband_sb = auxp.tile([128, NCH], fp32, tag="wband", bufs=3)
                nc.gpsimd.dma_start(
                    out=wband_sb[:, :], in_=wband_d[i].rearrange("(c p) -> p c", p=128)
                )
                lhs6 = auxp.tile([128, NCH, 6], fp16, tag="lhs6", bufs=3)
                nc.gpsimd.dma_start(
                    out=lhs6[:, :, 0:3],
                    in_=selfw_d[i].rearrange("(c p) r -> p c r", p=128),
                )

                # ---- banded Gram, upper-triangular blocks (fp16) ----
                gb = [
                    gbp.tile([128, S], fp16, tag="gb", bufs=8, name=f"gb{i}_{m}")
                    for m in range(NCH)
                ]
                for m in range(NCH):
                    w = S - 128 * m
                    g_ps = psp.tile([128, S], fp32, tag="g", bufs=2)
                    for j in range(NHB):
                        nc.tensor.matmul(
                            g_ps[:, 0:w],
                            xh[:, j, m * 128 : (m + 1) * 128],
                            xh[:, j, m * 128 : S],
                            start=(j == 0),
                            stop=(j == NHB - 1),
                        )
                    absd = workp.tile([128, S], fp32, tag="absd", bufs=3)
                    nc.scalar.activation(
                        out=absd[:, 0:w],
                        in_=iota_sb[:, m * 128 : S],
                        func=Act.Abs,
                        bias=negc_sb[:, m : m + 1],
                        scale=1.0,
                    )
                    nc.vector.scalar_tensor_tensor(
                        out=gb[m][:, m * 128 : S],
                        in0=absd[:, 0:w],
                        scalar=wband_sb[:, m : m + 1],
                        in1=g_ps[:, 0:w],
                        op0=Alu.is_le,
                        op1=Alu.mult,
                    )
                # lower-triangular blocks = transposes of the upper ones
                for mi in range(NCH):
                    for mj in range(mi + 1, NCH):
                        t_ps = psp.tile([128, 128], fp16, tag="gbt", bufs=2)
                        nc.tensor.transpose(
                            t_ps[:, :],
                            gb[mi][:, mj * 128 : (mj + 1) * 128],
                            eye_h[:, :],
                        )
                        nc.vector.tensor_copy(
                            gb[mj][:, mi * 128 : (mi + 1) * 128], t_ps[:, :]
                        )

                # ---- weighted column sums (rank-4 mask folded into PE) ----
                p12_ps = psp.tile([12, S], fp32, tag="p12", bufs=1)
                for c in range(NCH):
                    nc.tensor.matmul(
                        p12_ps[:, :],
                        v12_sb[:, c, :],
                        gb[c][:, :],
                        start=(c == 0),
                        stop=(c == NCH - 1),
                    )
                p12b = workp.tile([12, S], fp16, tag="p12b", bufs=3)
                nc.vector.tensor_mul(p12b[:, :], p12_ps[:, :], b12_sb[:, :])
                # P3^T per s-chunk: [12,128]^T @ e3 -> [128,3]
                p3t_ps = psp.tile([128, NCH, 3], fp32, tag="p3t", bufs=1)
                for c in range(NCH):
                    nc.tensor.matmul(
                        p3t_ps[:, c, :],
                        p12b[:, c * 128 : (c + 1) * 128],
                        e3_sb[:, :],
                        start=True,
                        stop=True,
                    )
                # u3^T = P3^T * C3^T straight into lhs6 cols 3:6 (fp16)
                nc.vector.tensor_mul(
                    lhs6[:, :, 3:6], p3t_ps[:, :, :], c3t_sb[:, :, :]
                )

                # ---- numerators [6,H]: lhs6^T @ x (fp16, fp32 accumulate) ----
                num_ps = psp.tile([6, H], fp32, tag="num", bufs=1)
                for n0, n1 in ((0, 512), (512, H)):
                    for c in range(NCH):
                        nc.tensor.matmul(
                            num_ps[:, n0:n1],
                            lhs6[:, c, :],
                            xt[:, c, n0:n1],
                            start=(c == 0),
                            stop=(c == NCH - 1),
                        )
                num_sb = workp.tile([6, H], fp32, tag="num_sb", bufs=2)
                nc.vector.tensor_copy(num_sb[:, :], num_ps[:, :])
                nc.sync.dma_start(out=out_d[i], in_=num_sb[:, :])

    nc.compile()
    return nc


def _get_nc():
    global _NC_CACHE
    if _NC_CACHE is None:
        _NC_CACHE = _build_nc()
    return _NC_CACHE


def _host_precompute(attention_mask, qa_ids, turn_ids):
    am = attention_mask.astype(np.float32)
    cq = ((qa_ids == 1) | (qa_ids == 2)).astype(np.float32) * am
    ca = ((qa_ids == 0) | (qa_ids == 2)).astype(np.float32) * am
    cn = (qa_ids == 3).astype(np.float32) * am
    sg = cq + ca + cn
    alpha = np.stack([sg, cq, ca, cn], axis=1)               # [B,4,S]
    sign = np.array([1.0, -1.0, -1.0, -1.0], np.float32)
    w2 = np.stack([ca + cn, cq + cn, cq + ca], axis=1)       # [B,3,S]
    V12 = (
        (w2[:, :, None, :] * alpha[:, None, :, :])
        .reshape(B, 12, S)
        .transpose(0, 2, 1)
        .copy()
    )                                                        # [B,S,12]
    B12 = np.tile(sign[None, :, None] * alpha, (1, 3, 1)) / 16.0  # [B,12,S], scaled for fp16 range
    C3T = np.stack([cq, ca, cn], axis=2)                     # [B,S,3]
    selfw = np.stack([cq * cq, ca * ca, cn * cn], axis=2)    # [B,S,3]
    negc = np.zeros((B, S), np.float32)
    wband = np.zeros((B, S), np.float32)
    for b in range(B):
        t = turn_ids[b]
        lo = np.searchsorted(t, t - VIEW_RANGE, side="left")
        hi = np.searchsorted(t, t + VIEW_RANGE, side="right") - 1
        negc[b] = -(lo + hi) / 2.0
        wband[b] = (hi - lo) / 2.0 + 0.25
    dens = np.stack([cq.sum(1), ca.sum(1), cn.sum(1)], axis=1) + AVG_EPS
    denc = (w2.sum(axis=2) + AVG_EPS) / 16.0  # matches the 1/16-scaled B12 cross path
    e3 = np.zeros((12, 3), np.float32)
    for j in range(3):
        e3[4 * j : 4 * j + 4, j] = 1.0
    return dict(
        V12=V12, B12=B12, C3T=C3T, selfw=selfw, negc=negc, wband=wband,
        dens=dens, denc=denc, e3=e3,
    )


def _host_finish(numer, labels, dens, denc):
    G = B // SAMPLE_NUMS
    self3 = (numer[:, 0:3, :].astype(np.float64) / dens[:, :, None]).astype(np.float32)
    cross3 = (numer[:, 3:6, :].astype(np.float64) / denc[:, :, None]).astype(np.float32)
    losses = []
    outputs = []
    for j in range(3):  # q, a, n
        xs = self3[:, j].reshape(G, SAMPLE_NUMS, H)
        xc = cross3[:, j].reshape(G, SAMPLE_NUMS, H)
        xs64 = xs.astype(np.float64)
        xc64 = xc.astype(np.float64)
        dot = np.sum(xs64 * xc64, axis=-1)
        nx = np.maximum(np.sqrt(np.sum(xs64 * xs64, axis=-1)), COS_EPS)
        ny = np.maximum(np.sqrt(np.sum(xc64 * xc64, axis=-1)), COS_EPS)
        c = (dot / (nx * ny)).astype(np.float32)
        c = np.where(c == np.float32(1.0), np.nan, c) / np.float32(TEMP)
        m = np.nanmax(c, axis=-1, keepdims=True)
        lse = np.log(np.sum(np.exp(c - m), axis=-1, keepdims=True)) + m
        lsm = c - lse
        losses.append(-np.nanmean(lsm * labels))
        outputs.append(xs[:, 0, :])
    stage_loss = np.float32((losses[1] + losses[0] + losses[2]) / 3.0)
    return stage_loss, outputs[0], outputs[1], outputs[2]


def _run_device(inputs, trace=False):
    from concourse.bass_utils import run_bass_kernel_spmd

    x = np.ascontiguousarray(np.asarray(inputs["self_output"], dtype=np.float32))
    x16 = x.astype(np.float16)
    aux = _host_precompute(
        np.asarray(inputs["attention_mask"], dtype=np.float32),
        np.asarray(inputs["qa_ids"]),
        np.asarray(inputs["turn_ids"]),
    )
    iota = np.arange(S, dtype=np.float32).reshape(1, S)
    in_maps = []
    for cidx in range(NCORES):
        sl = slice(cidx * PER, (cidx + 1) * PER)
        in_maps.append(
            {
                "x16": x16[sl],
                "v12": np.ascontiguousarray(aux["V12"][sl]).astype(np.float16),
                "b12": np.ascontiguousarray(aux["B12"][sl]),
                "c3t": np.ascontiguousarray(aux["C3T"][sl]),
                "selfw": np.ascontiguousarray(aux["selfw"][sl]).astype(np.float16),
                "negc": np.ascontiguousarray(aux["negc"][sl]),
                "wband": np.ascontiguousarray(aux["wband"][sl]),
                "iota": iota,
                "e3": aux["e3"].astype(np.float16),
            }
        )
    nc = _get_nc()
    res = run_bass_kernel_spmd(
        nc, in_maps, core_ids=list(range(NCORES)), trace=trace
    )
    numer = np.concatenate([res.results[c]["out"] for c in range(NCORES)], axis=0)
    return numer, aux, res


def kernel(**inputs):
    numer, aux, _ = _run_device(inputs)
    labels = np.asarray(inputs["labels"], dtype=np.float32)
    return _host_finish(numer, labels, aux["dens"], aux["denc"])


# revision 8
# speedup vs baseline: 1.4197x; 1.4197x over previous
"""Trainium2 Bass kernel for nn_Dial2vec (dialogue contrastive pretraining loss).

Strategy
--------
All three role-masked tensors are per-token scalar multiples of the same
hidden states: q_so[t] = cq[t]*x[t], etc.  Hence the role-masked cross scores
collapse onto ONE Gram matrix G = x @ x.T via a rank-4 outer-product mask
    K = sg sg^T - cq cq^T - ca ca^T - cn cn^T          (sg = cq+ca+cn)
and the cross outputs are only ever consumed through masked averages, which
turn the [S,S]@[S,H] matmuls into weighted vector-matrix products.

Per sample the device computes (fp16 operands, fp32 PSUM accumulation):
    G   = x16 x16^T   (upper-triangular blocks only; G is symmetric)
    Gb  = G * band    (band from sorted turn_ids: |iota - c_s| <= w_s,
                       fused DVE op; lower blocks = PE-transposed upper)
    P12 = V12^T Gb ;  P3^T = p12b^T E3 ;  u3^T = P3^T * C3^T
    numT[h,j] = x^T [selfw | u3^T]      (fp32, N=6 matmuls)

Everything runs in fp16 with fp32 accumulation (measured: loss rel-err
~2e-5, output tensors ~2e-4); the cross path carries a 1/16 scale for fp16
dynamic-range headroom.  Host does the cheap O(B*S) mask precompute and
the final divisions / cosine / log-softmax / nanmean on [8,9] tensors.

Sharding: data-parallel over samples, 72 = 8 cores x 9 samples.
"""

import numpy as np
import ml_dtypes

S = 512
H = 768
B = 72
NCORES = 8
PER = B // NCORES          # 9 samples per core
NCH = S // 128             # 4 partition chunks of the sequence
NHB = H // 128             # 6 hidden blocks
SAMPLE_NUMS = 9
VIEW_RANGE = 2
TEMP = 0.07
AVG_EPS = 1e-6
COS_EPS = 1e-8

_NC_CACHE = None


def _build_nc():
    import concourse.bacc as bacc
    import concourse.bass as bass
    import concourse.tile as tile
    import concourse.mybir as mybir
    from concourse.masks import make_identity

    fp32 = mybir.dt.float32
    fp16 = mybir.dt.float16
    Alu = mybir.AluOpType
    Act = mybir.ActivationFunctionType

    nc = bacc.Bacc("TRN2", target_bir_lowering=False, debug=False)
    x_d = nc.dram_tensor("x16", [PER, S, H], fp16, kind="ExternalInput").ap()
    v12_d = nc.dram_tensor("v12", [PER, S, 12], fp16, kind="ExternalInput").ap()
    b12_d = nc.dram_tensor("b12", [PER, 12, S], fp32, kind="ExternalInput").ap()
    c3t_d = nc.dram_tensor("c3t", [PER, S, 3], fp32, kind="ExternalInput").ap()
    selfw_d = nc.dram_tensor("selfw", [PER, S, 3], fp16, kind="ExternalInput").ap()
    negc_d = nc.dram_tensor("negc", [PER, S], fp32, kind="ExternalInput").ap()
    wband_d = nc.dram_tensor("wband", [PER, S], fp32, kind="ExternalInput").ap()
    iota_d = nc.dram_tensor("iota", [1, S], fp32, kind="ExternalInput").ap()
    e3_d = nc.dram_tensor("e3", [12, 3], fp16, kind="ExternalInput").ap()
    out_d = nc.dram_tensor("out", [PER, 6, H], fp32, kind="ExternalOutput").ap()

    with tile.TileContext(nc) as tc:
        with (
            tc.tile_pool(name="const", bufs=1) as constp,
            tc.tile_pool(name="xt", bufs=3) as xtp,
            tc.tile_pool(name="xh", bufs=3) as xhp,
            tc.tile_pool(name="gb", bufs=12) as gbp,
            tc.tile_pool(name="work", bufs=4) as workp,
            tc.tile_pool(name="aux", bufs=4) as auxp,
            tc.tile_pool(name="psum", bufs=1, space=bass.MemorySpace.PSUM) as psp,
        ):
            eye_h = constp.tile([128, 128], fp16, name="eye_h")
            make_identity(nc, eye_h[:, :])
            iota_sb = constp.tile([128, S], fp32, name="iota_sb")
            nc.sync.dma_start(
                out=iota_sb[:, :],
                in_=bass.AP(tensor=iota_d.tensor, offset=0, ap=[[0, 128], [1, S]]),
            )
            e3_sb = constp.tile([12, 3], fp16, name="e3_sb")
            nc.sync.dma_start(out=e3_sb[:, :], in_=e3_d)

            st = {}  # per-sample live tiles

            def loads(i):
                xt = xtp.tile([128, NCH, H], fp16, tag="xt", bufs=3, name=f"xt{i}")
                xi = x_d[i].rearrange("(c p) h -> p c h", p=128)
                for c in range(NCH):
                    nc.sync.dma_start(out=xt[:, c, :], in_=xi[:, c, :])
                # hidden-major layout via one xbar transpose from DRAM
                xh = xhp.tile([128, NHB, S], fp16, tag="xh", bufs=3, name=f"xh{i}")
                nc.scalar.dma_start_transpose(out=xh[:, :, :], in_=x_d[i])
                v12_sb = auxp.tile([128, NCH, 12], fp16, tag="v12", name=f"v12s{i}")
                nc.sync.dma_start(
                    out=v12_sb[:, :, :],
                    in_=v12_d[i].rearrange("(c p) r -> p c r", p=128),
                )
                b12_sb = auxp.tile([12, S], fp32, tag="b12", name=f"b12s{i}")
                nc.sync.dma_start(out=b12_sb[:, :], in_=b12_d[i])
                c3t_sb = auxp.tile([128, NCH, 3], fp32, tag="c3t", name=f"c3ts{i}")
                nc.sync.dma_start(
                    out=c3t_sb[:, :, :],
                    in_=c3t_d[i].rearrange("(c p) r -> p c r", p=128),
                )
                negc_sb = auxp.tile([128, NCH], fp32, tag="negc", name=f"negcs{i}")
                nc.sync.dma_start(
                    out=negc_sb[:, :], in_=negc_d[i].rearrange("(c p) -> p c", p=128)
                )
                wband_sb = auxp.tile([128, NCH], fp32, tag="wband", name=f"wbands{i}")
                nc.sync.dma_start(
                    out=wband_sb[:, :], in_=wband_d[i].rearrange("(c p) -> p c", p=128)
                )
                lhs6 = auxp.tile([128, NCH, 6], fp16, tag="lhs6", name=f"lhs6s{i}")
                nc.sync.dma_start(
                    out=lhs6[:, :, 0:3],
                    in_=selfw_d[i].rearrange("(c p) r -> p c r", p=128),
                )
                st[i] = dict(
                    xt=xt, xh=xh, v12=v12_sb, b12=b12_sb, c3t=c3t_sb,
                    negc=negc_sb, wband=wband_sb, lhs6=lhs6,
                )

            def gram(i):
                s = st[i]
                gb = [
                    gbp.tile([128, S], fp16, tag="gb", bufs=12, name=f"gb{i}_{m}")
                    for m in range(NCH)
                ]
                s["gb"] = gb
                for m in range(NCH):
                    w = S - 128 * m
                    g_ps = psp.tile(
                        [128, S], fp32, tag="g", bufs=2, name=f"g_ps{i}_{m}"
                    )
                    for j in range(NHB):
                        nc.tensor.matmul(
                            g_ps[:, 0:w],
                            s["xh"][:, j, m * 128 : (m + 1) * 128],
                            s["xh"][:, j, m * 128 : S],
                            start=(j == 0),
                            stop=(j == NHB - 1),
                        )
                    absd = workp.tile(
                        [128, S], fp32, tag="absd", bufs=4, name=f"absd{i}_{m}"
                    )
                    nc.scalar.activation(
                        out=absd[:, 0:w],
                        in_=iota_sb[:, m * 128 : S],
                        func=Act.Abs,
                        bias=s["negc"][:, m : m + 1],
                        scale=1.0,
                    )
                    nc.vector.scalar_tensor_tensor(
                        out=gb[m][:, m * 128 : S],
                        in0=absd[:, 0:w],
                        scalar=s["wband"][:, m : m + 1],
                        in1=g_ps[:, 0:w],
                        op0=Alu.is_le,
                        op1=Alu.mult,
                    )

            def tail_a(i):
                # lower-triangular Gb blocks, then the P12/P3 chain
                s = st[i]
                gb = s["gb"]
                for mi in range(NCH):
                    for mj in range(mi + 1, NCH):
                        t_ps = psp.tile(
                            [128, 128], fp16, tag="gbt", bufs=2,
                            name=f"t_ps{i}_{mi}{mj}",
                        )
                        nc.tensor.transpose(
                            t_ps[:, :],
                            gb[mi][:, mj * 128 : (mj + 1) * 128],
                            eye_h[:, :],
                        )
                        nc.vector.tensor_copy(
                            gb[mj][:, mi * 128 : (mi + 1) * 128], t_ps[:, :]
                        )
                p12_ps = psp.tile([12, S], fp32, tag="p12", bufs=1, name=f"p12_ps{i}")
                for c in range(NCH):
                    nc.tensor.matmul(
                        p12_ps[:, :],
                        s["v12"][:, c, :],
                        gb[c][:, :],
                        start=(c == 0),
                        stop=(c == NCH - 1),
                    )
                p12b = workp.tile([12, S], fp16, tag="p12b", bufs=4, name=f"p12b{i}")
                nc.vector.tensor_mul(p12b[:, :], p12_ps[:, :], s["b12"][:, :])
                p3t_ps = psp.tile(
                    [128, NCH, 3], fp32, tag="p3t", bufs=1, name=f"p3t_ps{i}"
                )
                for c in range(NCH):
                    nc.tensor.matmul(
                        p3t_ps[:, c, :],
                        p12b[:, c * 128 : (c + 1) * 128],
                        e3_sb[:, :],
                        start=True,
                        stop=True,
                    )
                nc.vector.tensor_mul(
                    s["lhs6"][:, :, 3:6], p3t_ps[:, :, :], s["c3t"][:, :, :]
                )

            def tail_b(i):
                s = st.pop(i)
                num_ps = psp.tile([6, H], fp32, tag="num", bufs=1, name=f"num_ps{i}")
                for n0, n1 in ((0, 512), (512, H)):
                    for c in range(NCH):
                        nc.tensor.matmul(
                            num_ps[:, n0:n1],
                            s["lhs6"][:, c, :],
                            s["xt"][:, c, n0:n1],
                            start=(c == 0),
                            stop=(c == NCH - 1),
                        )
                num_sb = workp.tile([6, H], fp32, tag="num_sb", bufs=2, name=f"num_sb{i}")
                nc.vector.tensor_copy(num_sb[:, :], num_ps[:, :])
                nc.scalar.dma_start(out=out_d[i], in_=num_sb[:, :])

            # 3-deep software pipeline keeps the tensor engine dense: sample
            # i's tail matmuls interleave with sample i+1/i+2's Gram phase.
            for i in range(PER):
                loads(i)
                gram(i)
                if i >= 1:
                    tail_a(i - 1)
                if i >= 2:
                    tail_b(i - 2)
            tail_a(PER - 1)
            tail_b(PER - 2)
            tail_b(PER - 1)

    nc.compile()
    return nc


def _get_nc():
    global _NC_CACHE
    if _NC_CACHE is None:
        _NC_CACHE = _build_nc()
    return _NC_CACHE


def _host_precompute(attention_mask, qa_ids, turn_ids):
    am = attention_mask.astype(np.float32)
    cq = ((qa_ids == 1) | (qa_ids == 2)).astype(np.float32) * am
    ca = ((qa_ids == 0) | (qa_ids == 2)).astype(np.float32) * am
    cn = (qa_ids == 3).astype(np.float32) * am
    sg = cq + ca + cn
    alpha = np.stack([sg, cq, ca, cn], axis=1)               # [B,4,S]
    sign = np.array([1.0, -1.0, -1.0, -1.0], np.float32)
    w2 = np.stack([ca + cn, cq + cn, cq + ca], axis=1)       # [B,3,S]
    V12 = (
        (w2[:, :, None, :] * alpha[:, None, :, :])
        .reshape(B, 12, S)
        .transpose(0, 2, 1)
        .copy()
    )                                                        # [B,S,12]
    B12 = np.tile(sign[None, :, None] * alpha, (1, 3, 1)) / 16.0  # [B,12,S], scaled for fp16 range
    C3T = np.stack([cq, ca, cn], axis=2)                     # [B,S,3]
    selfw = np.stack([cq * cq, ca * ca, cn * cn], axis=2)    # [B,S,3]
    negc = np.zeros((B, S), np.float32)
    wband = np.zeros((B, S), np.float32)
    for b in range(B):
        t = turn_ids[b]
        lo = np.searchsorted(t, t - VIEW_RANGE, side="left")
        hi = np.searchsorted(t, t + VIEW_RANGE, side="right") - 1
        negc[b] = -(lo + hi) / 2.0
        wband[b] = (hi - lo) / 2.0 + 0.25
    dens = np.stack([cq.sum(1), ca.sum(1), cn.sum(1)], axis=1) + AVG_EPS
    denc = (w2.sum(axis=2) + AVG_EPS) / 16.0  # matches the 1/16-scaled B12 cross path
    e3 = np.zeros((12, 3), np.float32)
    for j in range(3):
        e3[4 * j : 4 * j + 4, j] = 1.0
    return dict(
        V12=V12, B12=B12, C3T=C3T, selfw=selfw, negc=negc, wband=wband,
        dens=dens, denc=denc, e3=e3,
    )


def _host_finish(numer, labels, dens, denc):
    G = B // SAMPLE_NUMS
    self3 = (numer[:, 0:3, :].astype(np.float64) / dens[:, :, None]).astype(np.float32)
    cross3 = (numer[:, 3:6, :].astype(np.float64) / denc[:, :, None]).astype(np.float32)
    losses = []
    outputs = []
    for j in range(3):  # q, a, n
        xs = self3[:, j].reshape(G, SAMPLE_NUMS, H)
        xc = cross3[:, j].reshape(G, SAMPLE_NUMS, H)
        xs64 = xs.astype(np.float64)
        xc64 = xc.astype(np.float64)
        dot = np.sum(xs64 * xc64, axis=-1)
        nx = np.maximum(np.sqrt(np.sum(xs64 * xs64, axis=-1)), COS_EPS)
        ny = np.maximum(np.sqrt(np.sum(xc64 * xc64, axis=-1)), COS_EPS)
        c = (dot / (nx * ny)).astype(np.float32)
        c = np.where(c == np.float32(1.0), np.nan, c) / np.float32(TEMP)
        m = np.nanmax(c, axis=-1, keepdims=True)
        lse = np.log(np.sum(np.exp(c - m), axis=-1, keepdims=True)) + m
        lsm = c - lse
        losses.append(-np.nanmean(lsm * labels))
        outputs.append(xs[:, 0, :])
    stage_loss = np.float32((losses[1] + losses[0] + losses[2]) / 3.0)
    return stage_loss, outputs[0], outputs[1], outputs[2]


def _run_device(inputs, trace=False):
    from concourse.bass_utils import run_bass_kernel_spmd

    x = np.ascontiguousarray(np.asarray(inputs["self_output"], dtype=np.float32))
    x16 = x.astype(np.float16)
    aux = _host_precompute(
        np.asarray(inputs["attention_mask"], dtype=np.float32),
        np.asarray(inputs["qa_ids"]),
        np.asarray(inputs["turn_ids"]),
    )
    iota = np.arange(S, dtype=np.float32).reshape(1, S)
    in_maps = []
    for cidx in range(NCORES):
        sl = slice(cidx * PER, (cidx + 1) * PER)
        in_maps.append(
            {
                "x16": x16[sl],
                "v12": np.ascontiguousarray(aux["V12"][sl]).astype(np.float16),
                "b12": np.ascontiguousarray(aux["B12"][sl]),
                "c3t": np.ascontiguousarray(aux["C3T"][sl]),
                "selfw": np.ascontiguousarray(aux["selfw"][sl]).astype(np.float16),
                "negc": np.ascontiguousarray(aux["negc"][sl]),
                "wband": np.ascontiguousarray(aux["wband"][sl]),
                "iota": iota,
                "e3": aux["e3"].astype(np.float16),
            }
        )
    nc = _get_nc()
    res = run_bass_kernel_spmd(
        nc, in_maps, core_ids=list(range(NCORES)), trace=trace
    )
    numer = np.concatenate([res.results[c]["out"] for c in range(NCORES)], axis=0)
    return numer, aux, res


def kernel(**inputs):
    numer, aux, _ = _run_device(inputs)
    labels = np.asarray(inputs["labels"], dtype=np.float32)
    return _host_finish(numer, labels, aux["dens"], aux["denc"])


# revision 9
# speedup vs baseline: 1.4362x; 1.0116x over previous
"""Trainium2 Bass kernel for nn_Dial2vec (dialogue contrastive pretraining loss).

Strategy
--------
All three role-masked tensors are per-token scalar multiples of the same
hidden states: q_so[t] = cq[t]*x[t], etc.  Hence the role-masked cross scores
collapse onto ONE Gram matrix G = x @ x.T via a rank-4 outer-product mask
    K = sg sg^T - cq cq^T - ca ca^T - cn cn^T          (sg = cq+ca+cn)
and the cross outputs are only ever consumed through masked averages, which
turn the [S,S]@[S,H] matmuls into weighted vector-matrix products.

Per sample the device computes (fp16 operands, fp32 PSUM accumulation):
    G   = x16 x16^T   (upper-triangular blocks only; G is symmetric)
    Gb  = G * band    (band from sorted turn_ids: |iota - c_s| <= w_s,
                       fused DVE op; lower blocks = PE-transposed upper)
    P12 = V12^T Gb ;  P3^T = p12b^T E3 ;  u3^T = P3^T * C3^T
    numT[h,j] = x^T [selfw | u3^T]      (fp32, N=6 matmuls)

Everything runs in fp16 with fp32 accumulation (measured: loss rel-err
~2e-5, output tensors ~2e-4); the cross path carries a 1/16 scale for fp16
dynamic-range headroom.  Host does the cheap O(B*S) mask precompute and
the final divisions / cosine / log-softmax / nanmean on [8,9] tensors.

Sharding: data-parallel over samples, 72 = 8 cores x 9 samples.
"""

import numpy as np
import ml_dtypes

S = 512
H = 768
B = 72
NCORES = 8
PER = B // NCORES          # 9 samples per core
NCH = S // 128             # 4 partition chunks of the sequence
NHB = H // 128             # 6 hidden blocks
SAMPLE_NUMS = 9
VIEW_RANGE = 2
TEMP = 0.07
AVG_EPS = 1e-6
COS_EPS = 1e-8

_NC_CACHE = None


def _build_nc():
    import concourse.bacc as bacc
    import concourse.bass as bass
    import concourse.tile as tile
    import concourse.mybir as mybir
    from concourse.masks import make_identity

    fp32 = mybir.dt.float32
    fp16 = mybir.dt.float16
    Alu = mybir.AluOpType
    Act = mybir.ActivationFunctionType

    nc = bacc.Bacc("TRN2", target_bir_lowering=False, debug=False)
    x_d = nc.dram_tensor("x16", [PER, S, H], fp16, kind="ExternalInput").ap()
    v12_d = nc.dram_tensor("v12", [PER, S, 12], fp16, kind="ExternalInput").ap()
    b12_d = nc.dram_tensor("b12", [PER, 12, S], fp32, kind="ExternalInput").ap()
    aux5_d = nc.dram_tensor("aux5", [PER, S, 5], fp32, kind="ExternalInput").ap()
    selfw_d = nc.dram_tensor("selfw", [PER, S, 3], fp16, kind="ExternalInput").ap()
    iota_d = nc.dram_tensor("iota", [1, S], fp32, kind="ExternalInput").ap()
    e3_d = nc.dram_tensor("e3", [12, 3], fp16, kind="ExternalInput").ap()
    out_d = nc.dram_tensor("out", [PER, 6, H], fp32, kind="ExternalOutput").ap()

    with tile.TileContext(nc) as tc:
        with (
            tc.tile_pool(name="const", bufs=1) as constp,
            tc.tile_pool(name="xt", bufs=3) as xtp,
            tc.tile_pool(name="xh", bufs=3) as xhp,
            tc.tile_pool(name="gb", bufs=12) as gbp,
            tc.tile_pool(name="work", bufs=4) as workp,
            tc.tile_pool(name="aux", bufs=4) as auxp,
            tc.tile_pool(name="psum", bufs=1, space=bass.MemorySpace.PSUM) as psp,
        ):
            eye_h = constp.tile([128, 128], fp16, name="eye_h")
            make_identity(nc, eye_h[:, :])
            iota_sb = constp.tile([128, S], fp32, name="iota_sb")
            nc.sync.dma_start(
                out=iota_sb[:, :],
                in_=bass.AP(tensor=iota_d.tensor, offset=0, ap=[[0, 128], [1, S]]),
            )
            e3_sb = constp.tile([12, 3], fp16, name="e3_sb")
            nc.sync.dma_start(out=e3_sb[:, :], in_=e3_d)

            st = {}  # per-sample live tiles

            # all 9 xbar transposes up front on the otherwise-idle scalar
            # ring: they only read DRAM, and batching them avoids per-sample
            # xbar-mode transitions that act like DMA barriers
            for i in range(PER):
                xh = xhp.tile([128, NHB, S], fp16, tag="xh", bufs=PER, name=f"xh{i}")
                nc.scalar.dma_start_transpose(out=xh[:, :, :], in_=x_d[i])
                st[i] = dict(xh=xh)

            def loads(i):
                xt = xtp.tile([128, NCH, H], fp16, tag="xt", bufs=3, name=f"xt{i}")
                nc.sync.dma_start(
                    out=xt[:, :, :], in_=x_d[i].rearrange("(c p) h -> p c h", p=128)
                )
                v12_sb = auxp.tile([128, NCH, 12], fp16, tag="v12", name=f"v12s{i}")
                nc.sync.dma_start(
                    out=v12_sb[:, :, :],
                    in_=v12_d[i].rearrange("(c p) r -> p c r", p=128),
                )
                b12_sb = auxp.tile([12, S], fp32, tag="b12", name=f"b12s{i}")
                nc.sync.dma_start(out=b12_sb[:, :], in_=b12_d[i])
                aux5_sb = auxp.tile([128, NCH, 5], fp32, tag="aux5", name=f"aux5s{i}")
                nc.sync.dma_start(
                    out=aux5_sb[:, :, :],
                    in_=aux5_d[i].rearrange("(c p) r -> p c r", p=128),
                )
                lhs6 = auxp.tile([128, NCH, 6], fp16, tag="lhs6", name=f"lhs6s{i}")
                nc.sync.dma_start(
                    out=lhs6[:, :, 0:3],
                    in_=selfw_d[i].rearrange("(c p) r -> p c r", p=128),
                )
                st[i].update(
                    xt=xt, v12=v12_sb, b12=b12_sb, aux5=aux5_sb, lhs6=lhs6
                )

            def gram(i):
                s = st[i]
                gb = [
                    gbp.tile([128, S], fp16, tag="gb", bufs=12, name=f"gb{i}_{m}")
                    for m in range(NCH)
                ]
                s["gb"] = gb
                for m in range(NCH):
                    w = S - 128 * m
                    g_ps = psp.tile(
                        [128, S], fp32, tag="g", bufs=2, name=f"g_ps{i}_{m}"
                    )
                    for j in range(NHB):
                        nc.tensor.matmul(
                            g_ps[:, 0:w],
                            s["xh"][:, j, m * 128 : (m + 1) * 128],
                            s["xh"][:, j, m * 128 : S],
                            start=(j == 0),
                            stop=(j == NHB - 1),
                        )
                    absd = workp.tile(
                        [128, S], fp32, tag="absd", bufs=4, name=f"absd{i}_{m}"
                    )
                    nc.scalar.activation(
                        out=absd[:, 0:w],
                        in_=iota_sb[:, m * 128 : S],
                        func=Act.Abs,
                        bias=s["aux5"][:, m, 3:4],
                        scale=1.0,
                    )
                    nc.vector.scalar_tensor_tensor(
                        out=gb[m][:, m * 128 : S],
                        in0=absd[:, 0:w],
                        scalar=s["aux5"][:, m, 4:5],
                        in1=g_ps[:, 0:w],
                        op0=Alu.is_le,
                        op1=Alu.mult,
                    )

            def tail_a(i):
                # lower-triangular Gb blocks, then the P12/P3 chain
                s = st[i]
                gb = s["gb"]
                for mi in range(NCH):
                    for mj in range(mi + 1, NCH):
                        t_ps = psp.tile(
                            [128, 128], fp16, tag="gbt", bufs=2,
                            name=f"t_ps{i}_{mi}{mj}",
                        )
                        nc.tensor.transpose(
                            t_ps[:, :],
                            gb[mi][:, mj * 128 : (mj + 1) * 128],
                            eye_h[:, :],
                        )
                        nc.vector.tensor_copy(
                            gb[mj][:, mi * 128 : (mi + 1) * 128], t_ps[:, :]
                        )
                p12_ps = psp.tile([12, S], fp32, tag="p12", bufs=1, name=f"p12_ps{i}")
                for c in range(NCH):
                    nc.tensor.matmul(
                        p12_ps[:, :],
                        s["v12"][:, c, :],
                        gb[c][:, :],
                        start=(c == 0),
                        stop=(c == NCH - 1),
                    )
                p12b = workp.tile([12, S], fp16, tag="p12b", bufs=4, name=f"p12b{i}")
                nc.vector.tensor_mul(p12b[:, :], p12_ps[:, :], s["b12"][:, :])
                p3t_ps = psp.tile(
                    [128, NCH, 3], fp32, tag="p3t", bufs=1, name=f"p3t_ps{i}"
                )
                for c in range(NCH):
                    nc.tensor.matmul(
                        p3t_ps[:, c, :],
                        p12b[:, c * 128 : (c + 1) * 128],
                        e3_sb[:, :],
                        start=True,
                        stop=True,
                    )
                nc.vector.tensor_mul(
                    s["lhs6"][:, :, 3:6], p3t_ps[:, :, :], s["aux5"][:, :, 0:3]
                )

            def tail_b(i):
                s = st.pop(i)
                num_ps = psp.tile([6, H], fp32, tag="num", bufs=1, name=f"num_ps{i}")
                for n0, n1 in ((0, 512), (512, H)):
                    for c in range(NCH):
                        nc.tensor.matmul(
                            num_ps[:, n0:n1],
                            s["lhs6"][:, c, :],
                            s["xt"][:, c, n0:n1],
                            start=(c == 0),
                            stop=(c == NCH - 1),
                        )
                num_sb = workp.tile([6, H], fp32, tag="num_sb", bufs=2, name=f"num_sb{i}")
                nc.vector.tensor_copy(num_sb[:, :], num_ps[:, :])
                nc.sync.dma_start(out=out_d[i], in_=num_sb[:, :])

            # 3-deep software pipeline keeps the tensor engine dense: sample
            # i's tail matmuls interleave with sample i+1/i+2's Gram phase.
            for i in range(PER):
                loads(i)
                gram(i)
                if i >= 1:
                    tail_a(i - 1)
                if i >= 2:
                    tail_b(i - 2)
            tail_a(PER - 1)
            tail_b(PER - 2)
            tail_b(PER - 1)

    nc.compile()
    return nc


def _get_nc():
    global _NC_CACHE
    if _NC_CACHE is None:
        _NC_CACHE = _build_nc()
    return _NC_CACHE


def _host_precompute(attention_mask, qa_ids, turn_ids):
    am = attention_mask.astype(np.float32)
    cq = ((qa_ids == 1) | (qa_ids == 2)).astype(np.float32) * am
    ca = ((qa_ids == 0) | (qa_ids == 2)).astype(np.float32) * am
    cn = (qa_ids == 3).astype(np.float32) * am
    sg = cq + ca + cn
    alpha = np.stack([sg, cq, ca, cn], axis=1)               # [B,4,S]
    sign = np.array([1.0, -1.0, -1.0, -1.0], np.float32)
    w2 = np.stack([ca + cn, cq + cn, cq + ca], axis=1)       # [B,3,S]
    V12 = (
        (w2[:, :, None, :] * alpha[:, None, :, :])
        .reshape(B, 12, S)
        .transpose(0, 2, 1)
        .copy()
    )                                                        # [B,S,12]
    B12 = np.tile(sign[None, :, None] * alpha, (1, 3, 1)) / 16.0  # [B,12,S], scaled for fp16 range
    C3T = np.stack([cq, ca, cn], axis=2)                     # [B,S,3]
    selfw = np.stack([cq * cq, ca * ca, cn * cn], axis=2)    # [B,S,3]
    negc = np.zeros((B, S), np.float32)
    wband = np.zeros((B, S), np.float32)
    for b in range(B):
        t = turn_ids[b]
        lo = np.searchsorted(t, t - VIEW_RANGE, side="left")
        hi = np.searchsorted(t, t + VIEW_RANGE, side="right") - 1
        negc[b] = -(lo + hi) / 2.0
        wband[b] = (hi - lo) / 2.0 + 0.25
    dens = np.stack([cq.sum(1), ca.sum(1), cn.sum(1)], axis=1) + AVG_EPS
    denc = (w2.sum(axis=2) + AVG_EPS) / 16.0  # matches the 1/16-scaled B12 cross path
    e3 = np.zeros((12, 3), np.float32)
    for j in range(3):
        e3[4 * j : 4 * j + 4, j] = 1.0
    return dict(
        V12=V12, B12=B12, C3T=C3T, selfw=selfw, negc=negc, wband=wband,
        dens=dens, denc=denc, e3=e3,
    )


def _host_finish(numer, labels, dens, denc):
    G = B // SAMPLE_NUMS
    self3 = (numer[:, 0:3, :].astype(np.float64) / dens[:, :, None]).astype(np.float32)
    cross3 = (numer[:, 3:6, :].astype(np.float64) / denc[:, :, None]).astype(np.float32)
    losses = []
    outputs = []
    for j in range(3):  # q, a, n
        xs = self3[:, j].reshape(G, SAMPLE_NUMS, H)
        xc = cross3[:, j].reshape(G, SAMPLE_NUMS, H)
        xs64 = xs.astype(np.float64)
        xc64 = xc.astype(np.float64)
        dot = np.sum(xs64 * xc64, axis=-1)
        nx = np.maximum(np.sqrt(np.sum(xs64 * xs64, axis=-1)), COS_EPS)
        ny = np.maximum(np.sqrt(np.sum(xc64 * xc64, axis=-1)), COS_EPS)
        c = (dot / (nx * ny)).astype(np.float32)
        c = np.where(c == np.float32(1.0), np.nan, c) / np.float32(TEMP)
        m = np.nanmax(c, axis=-1, keepdims=True)
        lse = np.log(np.sum(np.exp(c - m), axis=-1, keepdims=True)) + m
        lsm = c - lse
        losses.append(-np.nanmean(lsm * labels))
        outputs.append(xs[:, 0, :])
    stage_loss = np.float32((losses[1] + losses[0] + losses[2]) / 3.0)
    return stage_loss, outputs[0], outputs[1], outputs[2]


def _run_device(inputs, trace=False):
    from concourse.bass_utils import run_bass_kernel_spmd

    x = np.ascontiguousarray(np.asarray(inputs["self_output"], dtype=np.float32))
    x16 = x.astype(np.float16)
    aux = _host_precompute(
        np.asarray(inputs["attention_mask"], dtype=np.float32),
        np.asarray(inputs["qa_ids"]),
        np.asarray(inputs["turn_ids"]),
    )
    iota = np.arange(S, dtype=np.float32).reshape(1, S)
    aux5 = np.concatenate(
        [aux["C3T"], aux["negc"][:, :, None], aux["wband"][:, :, None]], axis=2
    ).astype(np.float32)
    in_maps = []
    for cidx in range(NCORES):
        sl = slice(cidx * PER, (cidx + 1) * PER)
        in_maps.append(
            {
                "x16": x16[sl],
                "v12": np.ascontiguousarray(aux["V12"][sl]).astype(np.float16),
                "b12": np.ascontiguousarray(aux["B12"][sl]),
                "aux5": np.ascontiguousarray(aux5[sl]),
                "selfw": np.ascontiguousarray(aux["selfw"][sl]).astype(np.float16),
                "iota": iota,
                "e3": aux["e3"].astype(np.float16),
            }
        )
    nc = _get_nc()
    res = run_bass_kernel_spmd(
        nc, in_maps, core_ids=list(range(NCORES)), trace=trace
    )
    numer = np.concatenate([res.results[c]["out"] for c in range(NCORES)], axis=0)
    return numer, aux, res


def kernel(**inputs):
    numer, aux, _ = _run_device(inputs)
    labels = np.asarray(inputs["labels"], dtype=np.float32)
    return _host_finish(numer, labels, aux["dens"], aux["denc"])


# revision 10
# speedup vs baseline: 2.3810x; 1.6578x over previous
"""Trainium2 Bass kernel for nn_Dial2vec (dialogue contrastive pretraining loss).

Strategy
--------
All three role-masked tensors are per-token scalar multiples of the same
hidden states: q_so[t] = cq[t]*x[t], etc.  Hence the role-masked cross scores
collapse onto ONE Gram matrix G = x @ x.T via a rank-4 outer-product mask
    K = sg sg^T - cq cq^T - ca ca^T - cn cn^T          (sg = cq+ca+cn)
and the cross outputs are only ever consumed through masked averages, which
turn the [S,S]@[S,H] matmuls into weighted vector-matrix products.

Per sample the device computes (fp16 operands, fp32 PSUM accumulation):
    G   = x16 x16^T   (upper-triangular blocks only; G is symmetric)
    Gb  = G * band    (band from sorted turn_ids: |iota - c_s| <= w_s,
                       fused DVE op; lower blocks = PE-transposed upper)
    P12 = V12^T Gb ;  P3^T = p12b^T E3 ;  u3^T = P3^T * C3^T
    numT[h,j] = x^T [selfw | u3^T]      (fp32, N=6 matmuls)

Everything runs in fp16 with fp32 accumulation (measured: loss rel-err
~2e-5, output tensors ~2e-4); the cross path carries a 1/16 scale for fp16
dynamic-range headroom.  Host does the cheap O(B*S) mask precompute and
the final divisions / cosine / log-softmax / nanmean on [8,9] tensors.

Sharding: data-parallel over samples, 72 = 8 cores x 9 samples.
"""

import numpy as np
import ml_dtypes

S = 512
H = 768
B = 72
NCORES = 8
PER = B // NCORES          # 9 samples per core
NCH = S // 128             # 4 partition chunks of the sequence
NHB = H // 128             # 6 hidden blocks
SAMPLE_NUMS = 9
VIEW_RANGE = 2
TEMP = 0.07
AVG_EPS = 1e-6
COS_EPS = 1e-8

_NC_CACHE = None


def _build_nc():
    import concourse.bacc as bacc
    import concourse.bass as bass
    import concourse.tile as tile
    import concourse.mybir as mybir
    from concourse.masks import make_identity

    fp32 = mybir.dt.float32
    fp16 = mybir.dt.float16
    Alu = mybir.AluOpType
    Act = mybir.ActivationFunctionType

    nc = bacc.Bacc("TRN2", target_bir_lowering=False, debug=False)
    x_d = nc.dram_tensor("x16", [PER, S, H], fp16, kind="ExternalInput").ap()
    xhT_d = nc.dram_tensor("xhT", [PER, H, S], fp16, kind="ExternalInput").ap()
    v12_d = nc.dram_tensor("v12", [PER, S, 12], fp16, kind="ExternalInput").ap()
    b12_d = nc.dram_tensor("b12", [PER, 12, S], fp32, kind="ExternalInput").ap()
    aux5_d = nc.dram_tensor("aux5", [PER, S, 5], fp32, kind="ExternalInput").ap()
    selfw_d = nc.dram_tensor("selfw", [PER, S, 3], fp16, kind="ExternalInput").ap()
    iota_d = nc.dram_tensor("iota", [1, S], fp32, kind="ExternalInput").ap()
    e3_d = nc.dram_tensor("e3", [12, 3], fp16, kind="ExternalInput").ap()
    out_d = nc.dram_tensor("out", [PER, 6, H], fp32, kind="ExternalOutput").ap()

    with tile.TileContext(nc) as tc:
        with (
            tc.tile_pool(name="const", bufs=1) as constp,
            tc.tile_pool(name="xt", bufs=3) as xtp,
            tc.tile_pool(name="xh", bufs=3) as xhp,
            tc.tile_pool(name="gb", bufs=12) as gbp,
            tc.tile_pool(name="work", bufs=4) as workp,
            tc.tile_pool(name="aux", bufs=4) as auxp,
            tc.tile_pool(name="psum", bufs=1, space=bass.MemorySpace.PSUM) as psp,
        ):
            eye_h = constp.tile([128, 128], fp16, name="eye_h")
            make_identity(nc, eye_h[:, :])
            iota_sb = constp.tile([128, S], fp32, name="iota_sb")
            nc.sync.dma_start(
                out=iota_sb[:, :],
                in_=bass.AP(tensor=iota_d.tensor, offset=0, ap=[[0, 128], [1, S]]),
            )
            e3_sb = constp.tile([12, 3], fp16, name="e3_sb")
            nc.sync.dma_start(out=e3_sb[:, :], in_=e3_d)

            st = {}  # per-sample live tiles

            def loads(i):
                # hidden-major copy comes pre-transposed from the host
                xh = xhp.tile([128, NHB, S], fp16, tag="xh", bufs=3, name=f"xh{i}")
                nc.scalar.dma_start(
                    out=xh[:, :, :], in_=xhT_d[i].rearrange("(j p) t -> p j t", p=128)
                )
                xt = xtp.tile([128, NCH, H], fp16, tag="xt", bufs=3, name=f"xt{i}")
                nc.scalar.dma_start(
                    out=xt[:, :, :], in_=x_d[i].rearrange("(c p) h -> p c h", p=128)
                )
                st[i] = dict(xh=xh)
                v12_sb = auxp.tile([128, NCH, 12], fp16, tag="v12", name=f"v12s{i}")
                nc.sync.dma_start(
                    out=v12_sb[:, :, :],
                    in_=v12_d[i].rearrange("(c p) r -> p c r", p=128),
                )
                b12_sb = auxp.tile([12, S], fp32, tag="b12", name=f"b12s{i}")
                nc.sync.dma_start(out=b12_sb[:, :], in_=b12_d[i])
                aux5_sb = auxp.tile([128, NCH, 5], fp32, tag="aux5", name=f"aux5s{i}")
                nc.sync.dma_start(
                    out=aux5_sb[:, :, :],
                    in_=aux5_d[i].rearrange("(c p) r -> p c r", p=128),
                )
                lhs6 = auxp.tile([128, NCH, 6], fp16, tag="lhs6", name=f"lhs6s{i}")
                nc.sync.dma_start(
                    out=lhs6[:, :, 0:3],
                    in_=selfw_d[i].rearrange("(c p) r -> p c r", p=128),
                )
                st[i].update(
                    xt=xt, v12=v12_sb, b12=b12_sb, aux5=aux5_sb, lhs6=lhs6
                )

            def gram(i):
                s = st[i]
                gb = [
                    gbp.tile([128, S], fp16, tag="gb", bufs=12, name=f"gb{i}_{m}")
                    for m in range(NCH)
                ]
                s["gb"] = gb
                for m in range(NCH):
                    w = S - 128 * m
                    g_ps = psp.tile(
                        [128, S], fp32, tag="g", bufs=2, name=f"g_ps{i}_{m}"
                    )
                    for j in range(NHB):
                        nc.tensor.matmul(
                            g_ps[:, 0:w],
                            s["xh"][:, j, m * 128 : (m + 1) * 128],
                            s["xh"][:, j, m * 128 : S],
                            start=(j == 0),
                            stop=(j == NHB - 1),
                        )
                    absd = workp.tile(
                        [128, S], fp32, tag="absd", bufs=4, name=f"absd{i}_{m}"
                    )
                    nc.scalar.activation(
                        out=absd[:, 0:w],
                        in_=iota_sb[:, m * 128 : S],
                        func=Act.Abs,
                        bias=s["aux5"][:, m, 3:4],
                        scale=1.0,
                    )
                    nc.vector.scalar_tensor_tensor(
                        out=gb[m][:, m * 128 : S],
                        in0=absd[:, 0:w],
                        scalar=s["aux5"][:, m, 4:5],
                        in1=g_ps[:, 0:w],
                        op0=Alu.is_le,
                        op1=Alu.mult,
                    )

            def tail_a(i):
                # lower-triangular Gb blocks, then the P12/P3 chain
                s = st[i]
                gb = s["gb"]
                for mi in range(NCH):
                    for mj in range(mi + 1, NCH):
                        t_ps = psp.tile(
                            [128, 128], fp16, tag="gbt", bufs=2,
                            name=f"t_ps{i}_{mi}{mj}",
                        )
                        nc.tensor.transpose(
                            t_ps[:, :],
                            gb[mi][:, mj * 128 : (mj + 1) * 128],
                            eye_h[:, :],
                        )
                        nc.vector.tensor_copy(
                            gb[mj][:, mi * 128 : (mi + 1) * 128], t_ps[:, :]
                        )
                p12_ps = psp.tile([12, S], fp32, tag="p12", bufs=1, name=f"p12_ps{i}")
                for c in range(NCH):
                    nc.tensor.matmul(
                        p12_ps[:, :],
                        s["v12"][:, c, :],
                        gb[c][:, :],
                        start=(c == 0),
                        stop=(c == NCH - 1),
                    )
                p12b = workp.tile([12, S], fp16, tag="p12b", bufs=4, name=f"p12b{i}")
                nc.vector.tensor_mul(p12b[:, :], p12_ps[:, :], s["b12"][:, :])
                p3t_ps = psp.tile(
                    [128, NCH, 3], fp32, tag="p3t", bufs=1, name=f"p3t_ps{i}"
                )
                for c in range(NCH):
                    nc.tensor.matmul(
                        p3t_ps[:, c, :],
                        p12b[:, c * 128 : (c + 1) * 128],
                        e3_sb[:, :],
                        start=True,
                        stop=True,
                    )
                nc.vector.tensor_mul(
                    s["lhs6"][:, :, 3:6], p3t_ps[:, :, :], s["aux5"][:, :, 0:3]
                )

            def tail_b(i):
                s = st.pop(i)
                num_ps = psp.tile([6, H], fp32, tag="num", bufs=1, name=f"num_ps{i}")
                for n0, n1 in ((0, 512), (512, H)):
                    for c in range(NCH):
                        nc.tensor.matmul(
                            num_ps[:, n0:n1],
                            s["lhs6"][:, c, :],
                            s["xt"][:, c, n0:n1],
                            start=(c == 0),
                            stop=(c == NCH - 1),
                        )
                num_sb = workp.tile([6, H], fp32, tag="num_sb", bufs=2, name=f"num_sb{i}")
                nc.vector.tensor_copy(num_sb[:, :], num_ps[:, :])
                nc.sync.dma_start(out=out_d[i], in_=num_sb[:, :])

            # 3-deep software pipeline keeps the tensor engine dense: sample
            # i's tail matmuls interleave with sample i+1/i+2's Gram phase.
            for i in range(PER):
                loads(i)
                gram(i)
                if i >= 1:
                    tail_a(i - 1)
                if i >= 2:
                    tail_b(i - 2)
            tail_a(PER - 1)
            tail_b(PER - 2)
            tail_b(PER - 1)

    nc.compile()
    return nc


def _get_nc():
    global _NC_CACHE
    if _NC_CACHE is None:
        _NC_CACHE = _build_nc()
    return _NC_CACHE


def _host_precompute(attention_mask, qa_ids, turn_ids):
    am = attention_mask.astype(np.float32)
    cq = ((qa_ids == 1) | (qa_ids == 2)).astype(np.float32) * am
    ca = ((qa_ids == 0) | (qa_ids == 2)).astype(np.float32) * am
    cn = (qa_ids == 3).astype(np.float32) * am
    sg = cq + ca + cn
    alpha = np.stack([sg, cq, ca, cn], axis=1)               # [B,4,S]
    sign = np.array([1.0, -1.0, -1.0, -1.0], np.float32)
    w2 = np.stack([ca + cn, cq + cn, cq + ca], axis=1)       # [B,3,S]
    V12 = (
        (w2[:, :, None, :] * alpha[:, None, :, :])
        .reshape(B, 12, S)
        .transpose(0, 2, 1)
        .copy()
    )                                                        # [B,S,12]
    B12 = np.tile(sign[None, :, None] * alpha, (1, 3, 1)) / 16.0  # [B,12,S], scaled for fp16 range
    C3T = np.stack([cq, ca, cn], axis=2)                     # [B,S,3]
    selfw = np.stack([cq * cq, ca * ca, cn * cn], axis=2)    # [B,S,3]
    negc = np.zeros((B, S), np.float32)
    wband = np.zeros((B, S), np.float32)
    for b in range(B):
        t = turn_ids[b]
        lo = np.searchsorted(t, t - VIEW_RANGE, side="left")
        hi = np.searchsorted(t, t + VIEW_RANGE, side="right") - 1
        negc[b] = -(lo + hi) / 2.0
        wband[b] = (hi - lo) / 2.0 + 0.25
    dens = np.stack([cq.sum(1), ca.sum(1), cn.sum(1)], axis=1) + AVG_EPS
    denc = (w2.sum(axis=2) + AVG_EPS) / 16.0  # matches the 1/16-scaled B12 cross path
    e3 = np.zeros((12, 3), np.float32)
    for j in range(3):
        e3[4 * j : 4 * j + 4, j] = 1.0
    return dict(
        V12=V12, B12=B12, C3T=C3T, selfw=selfw, negc=negc, wband=wband,
        dens=dens, denc=denc, e3=e3,
    )


def _host_finish(numer, labels, dens, denc):
    G = B // SAMPLE_NUMS
    self3 = (numer[:, 0:3, :].astype(np.float64) / dens[:, :, None]).astype(np.float32)
    cross3 = (numer[:, 3:6, :].astype(np.float64) / denc[:, :, None]).astype(np.float32)
    losses = []
    outputs = []
    for j in range(3):  # q, a, n
        xs = self3[:, j].reshape(G, SAMPLE_NUMS, H)
        xc = cross3[:, j].reshape(G, SAMPLE_NUMS, H)
        xs64 = xs.astype(np.float64)
        xc64 = xc.astype(np.float64)
        dot = np.sum(xs64 * xc64, axis=-1)
        nx = np.maximum(np.sqrt(np.sum(xs64 * xs64, axis=-1)), COS_EPS)
        ny = np.maximum(np.sqrt(np.sum(xc64 * xc64, axis=-1)), COS_EPS)
        c = (dot / (nx * ny)).astype(np.float32)
        c = np.where(c == np.float32(1.0), np.nan, c) / np.float32(TEMP)
        m = np.nanmax(c, axis=-1, keepdims=True)
        lse = np.log(np.sum(np.exp(c - m), axis=-1, keepdims=True)) + m
        lsm = c - lse
        losses.append(-np.nanmean(lsm * labels))
        outputs.append(xs[:, 0, :])
    stage_loss = np.float32((losses[1] + losses[0] + losses[2]) / 3.0)
    return stage_loss, outputs[0], outputs[1], outputs[2]


def _run_device(inputs, trace=False):
    from concourse.bass_utils import run_bass_kernel_spmd

    x = np.ascontiguousarray(np.asarray(inputs["self_output"], dtype=np.float32))
    x16 = x.astype(np.float16)
    xhT = np.ascontiguousarray(x16.transpose(0, 2, 1))
    aux = _host_precompute(
        np.asarray(inputs["attention_mask"], dtype=np.float32),
        np.asarray(inputs["qa_ids"]),
        np.asarray(inputs["turn_ids"]),
    )
    iota = np.arange(S, dtype=np.float32).reshape(1, S)
    aux5 = np.concatenate(
        [aux["C3T"], aux["negc"][:, :, None], aux["wband"][:, :, None]], axis=2
    ).astype(np.float32)
    in_maps = []
    for cidx in range(NCORES):
        sl = slice(cidx * PER, (cidx + 1) * PER)
        in_maps.append(
            {
                "x16": x16[sl],
                "xhT": xhT[sl],
                "v12": np.ascontiguousarray(aux["V12"][sl]).astype(np.float16),
                "b12": np.ascontiguousarray(aux["B12"][sl]),
                "aux5": np.ascontiguousarray(aux5[sl]),
                "selfw": np.ascontiguousarray(aux["selfw"][sl]).astype(np.float16),
                "iota": iota,
                "e3": aux["e3"].astype(np.float16),
            }
        )
    nc = _get_nc()
    res = run_bass_kernel_spmd(
        nc, in_maps, core_ids=list(range(NCORES)), trace=trace
    )
    numer = np.concatenate([res.results[c]["out"] for c in range(NCORES)], axis=0)
    return numer, aux, res


def kernel(**inputs):
    numer, aux, _ = _run_device(inputs)
    labels = np.asarray(inputs["labels"], dtype=np.float32)
    return _host_finish(numer, labels, aux["dens"], aux["denc"])


# revision 12
# speedup vs baseline: 2.4303x; 1.0207x over previous
"""Trainium2 Bass kernel for nn_Dial2vec (dialogue contrastive pretraining loss).

Strategy
--------
All three role-masked tensors are per-token scalar multiples of the same
hidden states: q_so[t] = cq[t]*x[t], etc.  Hence the role-masked cross scores
collapse onto ONE Gram matrix G = x @ x.T via a rank-4 outer-product mask
    K = sg sg^T - cq cq^T - ca ca^T - cn cn^T          (sg = cq+ca+cn)
and the cross outputs are only ever consumed through masked averages, which
turn the [S,S]@[S,H] matmuls into weighted vector-matrix products.

Per sample the device computes (fp16 operands, fp32 PSUM accumulation):
    G   = x16 x16^T   (upper-triangular blocks only; G is symmetric)
    Gb  = G * band    (band from sorted turn_ids: |iota - c_s| <= w_s,
                       fused DVE op; lower blocks = PE-transposed upper)
    P12 = V12^T Gb ;  P3^T = p12b^T E3 ;  u3^T = P3^T * C3^T
    numT[h,j] = x^T [selfw | u3^T]      (fp32, N=6 matmuls)

Everything runs in fp16 with fp32 accumulation (measured: loss rel-err
~2e-5, output tensors ~2e-4); the cross path carries a 1/16 scale for fp16
dynamic-range headroom.  Host does the cheap O(B*S) mask precompute and
the final divisions / cosine / log-softmax / nanmean on [8,9] tensors.

Sharding: data-parallel over samples, 72 = 8 cores x 9 samples.
"""

import numpy as np
import ml_dtypes

S = 512
H = 768
B = 72
NCORES = 8
PER = B // NCORES          # 9 samples per core
NCH = S // 128             # 4 partition chunks of the sequence
NHB = H // 128             # 6 hidden blocks
SAMPLE_NUMS = 9
VIEW_RANGE = 2
TEMP = 0.07
AVG_EPS = 1e-6
COS_EPS = 1e-8

_NC_CACHE = None


def _build_nc():
    import concourse.bacc as bacc
    import concourse.bass as bass
    import concourse.tile as tile
    import concourse.mybir as mybir
    from concourse.masks import make_identity

    fp32 = mybir.dt.float32
    fp16 = mybir.dt.float16
    Alu = mybir.AluOpType
    Act = mybir.ActivationFunctionType

    nc = bacc.Bacc("TRN2", target_bir_lowering=False, debug=False)
    x_d = nc.dram_tensor("x16", [PER, S, H], fp16, kind="ExternalInput").ap()
    xhT_d = nc.dram_tensor("xhT", [PER, H, S], fp16, kind="ExternalInput").ap()
    v12_d = nc.dram_tensor("v12", [PER, S, 12], fp16, kind="ExternalInput").ap()
    b12_d = nc.dram_tensor("b12", [PER, 12, S], fp32, kind="ExternalInput").ap()
    aux5_d = nc.dram_tensor("aux5", [PER, S, 5], fp32, kind="ExternalInput").ap()
    selfw_d = nc.dram_tensor("selfw", [PER, S, 3], fp16, kind="ExternalInput").ap()
    e3_d = nc.dram_tensor("e3", [12, 3], fp16, kind="ExternalInput").ap()
    out_d = nc.dram_tensor("out", [PER, 6, H], fp32, kind="ExternalOutput").ap()

    with tile.TileContext(nc) as tc:
        with (
            tc.tile_pool(name="const", bufs=1) as constp,
            tc.tile_pool(name="xt", bufs=3) as xtp,
            tc.tile_pool(name="xh", bufs=3) as xhp,
            tc.tile_pool(name="gb", bufs=12) as gbp,
            tc.tile_pool(name="work", bufs=4) as workp,
            tc.tile_pool(name="aux", bufs=4) as auxp,
            tc.tile_pool(name="psum", bufs=1, space=bass.MemorySpace.PSUM) as psp,
        ):
            eye_h = constp.tile([128, 128], fp16, name="eye_h")
            make_identity(nc, eye_h[:, :])
            iota_sb = constp.tile([128, S], fp32, name="iota_sb")
            nc.gpsimd.iota(
                iota_sb[:, :], pattern=[[1, S]], base=0, channel_multiplier=0,
                allow_small_or_imprecise_dtypes=True,
            )
            e3_sb = constp.tile([12, 3], fp16, name="e3_sb")
            nc.sync.dma_start(out=e3_sb[:, :], in_=e3_d)

            st = {}  # per-sample live tiles

            def loads(i):
                # hidden-major copy comes pre-transposed from the host;
                # halves go on both HWDGE rings to cut arrival latency
                xh = xhp.tile([128, NHB, S], fp16, tag="xh", bufs=3, name=f"xh{i}")
                xhi = xhT_d[i].rearrange("(j p) t -> p j t", p=128)
                nc.scalar.dma_start(out=xh[:, 0:3, :], in_=xhi[:, 0:3, :])
                nc.sync.dma_start(out=xh[:, 3:6, :], in_=xhi[:, 3:6, :])
                xt = xtp.tile([128, NCH, H], fp16, tag="xt", bufs=3, name=f"xt{i}")
                xti = x_d[i].rearrange("(c p) h -> p c h", p=128)
                nc.scalar.dma_start(out=xt[:, 0:2, :], in_=xti[:, 0:2, :])
                nc.sync.dma_start(out=xt[:, 2:4, :], in_=xti[:, 2:4, :])
                st[i] = dict(xh=xh)
                v12_sb = auxp.tile([128, NCH, 12], fp16, tag="v12", name=f"v12s{i}")
                nc.sync.dma_start(
                    out=v12_sb[:, :, :],
                    in_=v12_d[i].rearrange("(c p) r -> p c r", p=128),
                )
                b12_sb = auxp.tile([12, S], fp32, tag="b12", name=f"b12s{i}")
                nc.sync.dma_start(out=b12_sb[:, :], in_=b12_d[i])
                aux5_sb = auxp.tile([128, NCH, 5], fp32, tag="aux5", name=f"aux5s{i}")
                nc.sync.dma_start(
                    out=aux5_sb[:, :, :],
                    in_=aux5_d[i].rearrange("(c p) r -> p c r", p=128),
                )
                lhs6 = auxp.tile([128, NCH, 6], fp16, tag="lhs6", name=f"lhs6s{i}")
                nc.sync.dma_start(
                    out=lhs6[:, :, 0:3],
                    in_=selfw_d[i].rearrange("(c p) r -> p c r", p=128),
                )
                st[i].update(
                    xt=xt, v12=v12_sb, b12=b12_sb, aux5=aux5_sb, lhs6=lhs6
                )

            def gram(i):
                s = st[i]
                gb = [
                    gbp.tile([128, S], fp16, tag="gb", bufs=12, name=f"gb{i}_{m}")
                    for m in range(NCH)
                ]
                s["gb"] = gb
                for m in range(NCH):
                    w = S - 128 * m
                    g_ps = psp.tile(
                        [128, S], fp32, tag="g", bufs=2, name=f"g_ps{i}_{m}"
                    )
                    for j in range(NHB):
                        nc.tensor.matmul(
                            g_ps[:, 0:w],
                            s["xh"][:, j, m * 128 : (m + 1) * 128],
                            s["xh"][:, j, m * 128 : S],
                            start=(j == 0),
                            stop=(j == NHB - 1),
                        )
                    absd = workp.tile(
                        [128, S], fp32, tag="absd", bufs=4, name=f"absd{i}_{m}"
                    )
                    nc.scalar.activation(
                        out=absd[:, 0:w],
                        in_=iota_sb[:, m * 128 : S],
                        func=Act.Abs,
                        bias=s["aux5"][:, m, 3:4],
                        scale=1.0,
                    )
                    nc.vector.scalar_tensor_tensor(
                        out=gb[m][:, m * 128 : S],
                        in0=absd[:, 0:w],
                        scalar=s["aux5"][:, m, 4:5],
                        in1=g_ps[:, 0:w],
                        op0=Alu.is_le,
                        op1=Alu.mult,
                    )

            def tail_a(i):
                # lower-triangular Gb blocks, then the P12/P3 chain
                s = st[i]
                gb = s["gb"]
                for mi in range(NCH):
                    for mj in range(mi + 1, NCH):
                        t_ps = psp.tile(
                            [128, 128], fp16, tag="gbt", bufs=2,
                            name=f"t_ps{i}_{mi}{mj}",
                        )
                        nc.tensor.transpose(
                            t_ps[:, :],
                            gb[mi][:, mj * 128 : (mj + 1) * 128],
                            eye_h[:, :],
                        )
                        nc.scalar.copy(
                            gb[mj][:, mi * 128 : (mi + 1) * 128], t_ps[:, :]
                        )
                p12_ps = psp.tile([12, S], fp32, tag="p12", bufs=1, name=f"p12_ps{i}")
                for c in range(NCH):
                    nc.tensor.matmul(
                        p12_ps[:, :],
                        s["v12"][:, c, :],
                        gb[c][:, :],
                        start=(c == 0),
                        stop=(c == NCH - 1),
                    )
                p12b = workp.tile([12, S], fp16, tag="p12b", bufs=4, name=f"p12b{i}")
                nc.vector.tensor_mul(p12b[:, :], p12_ps[:, :], s["b12"][:, :])
                p3t_ps = psp.tile(
                    [128, NCH, 3], fp32, tag="p3t", bufs=1, name=f"p3t_ps{i}"
                )
                for c in range(NCH):
                    nc.tensor.matmul(
                        p3t_ps[:, c, :],
                        p12b[:, c * 128 : (c + 1) * 128],
                        e3_sb[:, :],
                        start=True,
                        stop=True,
                    )
                nc.vector.tensor_mul(
                    s["lhs6"][:, :, 3:6], p3t_ps[:, :, :], s["aux5"][:, :, 0:3]
                )

            def tail_b(i):
                s = st.pop(i)
                num_ps = psp.tile([6, H], fp32, tag="num", bufs=1, name=f"num_ps{i}")
                for n0, n1 in ((0, 512), (512, H)):
                    for c in range(NCH):
                        nc.tensor.matmul(
                            num_ps[:, n0:n1],
                            s["lhs6"][:, c, :],
                            s["xt"][:, c, n0:n1],
                            start=(c == 0),
                            stop=(c == NCH - 1),
                        )
                num_sb = workp.tile([6, H], fp32, tag="num_sb", bufs=2, name=f"num_sb{i}")
                nc.vector.tensor_copy(num_sb[:, :], num_ps[:, :])
                nc.sync.dma_start(out=out_d[i], in_=num_sb[:, :])

            # 3-deep software pipeline keeps the tensor engine dense: sample
            # i's tail matmuls interleave with sample i+1/i+2's Gram phase.
            for i in range(PER):
                loads(i)
                gram(i)
                if i >= 1:
                    tail_a(i - 1)
                if i >= 2:
                    tail_b(i - 2)
            tail_a(PER - 1)
            tail_b(PER - 2)
            tail_b(PER - 1)

    nc.compile()
    return nc


def _get_nc():
    global _NC_CACHE
    if _NC_CACHE is None:
        _NC_CACHE = _build_nc()
    return _NC_CACHE


def _host_precompute(attention_mask, qa_ids, turn_ids):
    am = attention_mask.astype(np.float32)
    cq = ((qa_ids == 1) | (qa_ids == 2)).astype(np.float32) * am
    ca = ((qa_ids == 0) | (qa_ids == 2)).astype(np.float32) * am
    cn = (qa_ids == 3).astype(np.float32) * am
    sg = cq + ca + cn
    alpha = np.stack([sg, cq, ca, cn], axis=1)               # [B,4,S]
    sign = np.array([1.0, -1.0, -1.0, -1.0], np.float32)
    w2 = np.stack([ca + cn, cq + cn, cq + ca], axis=1)       # [B,3,S]
    V12 = (
        (w2[:, :, None, :] * alpha[:, None, :, :])
        .reshape(B, 12, S)
        .transpose(0, 2, 1)
        .copy()
    )                                                        # [B,S,12]
    B12 = np.tile(sign[None, :, None] * alpha, (1, 3, 1)) / 16.0  # [B,12,S], scaled for fp16 range
    C3T = np.stack([cq, ca, cn], axis=2)                     # [B,S,3]
    selfw = np.stack([cq * cq, ca * ca, cn * cn], axis=2)    # [B,S,3]
    negc = np.zeros((B, S), np.float32)
    wband = np.zeros((B, S), np.float32)
    for b in range(B):
        t = turn_ids[b]
        lo = np.searchsorted(t, t - VIEW_RANGE, side="left")
        hi = np.searchsorted(t, t + VIEW_RANGE, side="right") - 1
        negc[b] = -(lo + hi) / 2.0
        wband[b] = (hi - lo) / 2.0 + 0.25
    dens = np.stack([cq.sum(1), ca.sum(1), cn.sum(1)], axis=1) + AVG_EPS
    denc = (w2.sum(axis=2) + AVG_EPS) / 16.0  # matches the 1/16-scaled B12 cross path
    e3 = np.zeros((12, 3), np.float32)
    for j in range(3):
        e3[4 * j : 4 * j + 4, j] = 1.0
    return dict(
        V12=V12, B12=B12, C3T=C3T, selfw=selfw, negc=negc, wband=wband,
        dens=dens, denc=denc, e3=e3,
    )


def _host_finish(numer, labels, dens, denc):
    G = B // SAMPLE_NUMS
    self3 = (numer[:, 0:3, :].astype(np.float64) / dens[:, :, None]).astype(np.float32)
    cross3 = (numer[:, 3:6, :].astype(np.float64) / denc[:, :, None]).astype(np.float32)
    losses = []
    outputs = []
    for j in range(3):  # q, a, n
        xs = self3[:, j].reshape(G, SAMPLE_NUMS, H)
        xc = cross3[:, j].reshape(G, SAMPLE_NUMS, H)
        xs64 = xs.astype(np.float64)
        xc64 = xc.astype(np.float64)
        dot = np.sum(xs64 * xc64, axis=-1)
        nx = np.maximum(np.sqrt(np.sum(xs64 * xs64, axis=-1)), COS_EPS)
        ny = np.maximum(np.sqrt(np.sum(xc64 * xc64, axis=-1)), COS_EPS)
        c = (dot / (nx * ny)).astype(np.float32)
        c = np.where(c == np.float32(1.0), np.nan, c) / np.float32(TEMP)
        m = np.nanmax(c, axis=-1, keepdims=True)
        lse = np.log(np.sum(np.exp(c - m), axis=-1, keepdims=True)) + m
        lsm = c - lse
        losses.append(-np.nanmean(lsm * labels))
        outputs.append(xs[:, 0, :])
    stage_loss = np.float32((losses[1] + losses[0] + losses[2]) / 3.0)
    return stage_loss, outputs[0], outputs[1], outputs[2]


def _run_device(inputs, trace=False):
    from concourse.bass_utils import run_bass_kernel_spmd

    x = np.ascontiguousarray(np.asarray(inputs["self_output"], dtype=np.float32))
    x16 = x.astype(np.float16)
    xhT = np.ascontiguousarray(x16.transpose(0, 2, 1))
    aux = _host_precompute(
        np.asarray(inputs["attention_mask"], dtype=np.float32),
        np.asarray(inputs["qa_ids"]),
        np.asarray(inputs["turn_ids"]),
    )
    aux5 = np.concatenate(
        [aux["C3T"], aux["negc"][:, :, None], aux["wband"][:, :, None]], axis=2
    ).astype(np.float32)
    in_maps = []
    for cidx in range(NCORES):
        sl = slice(cidx * PER, (cidx + 1) * PER)
        in_maps.append(
            {
                "x16": x16[sl],
                "xhT": xhT[sl],
                "v12": np.ascontiguousarray(aux["V12"][sl]).astype(np.float16),
                "b12": np.ascontiguousarray(aux["B12"][sl]),
                "aux5": np.ascontiguousarray(aux5[sl]),
                "selfw": np.ascontiguousarray(aux["selfw"][sl]).astype(np.float16),
                "e3": aux["e3"].astype(np.float16),
            }
        )
    nc = _get_nc()
    res = run_bass_kernel_spmd(
        nc, in_maps, core_ids=list(range(NCORES)), trace=trace
    )
    numer = np.concatenate([res.results[c]["out"] for c in range(NCORES)], axis=0)
    return numer, aux, res


def kernel(**inputs):
    numer, aux, _ = _run_device(inputs)
    labels = np.asarray(inputs["labels"], dtype=np.float32)
    return _host_finish(numer, labels, aux["dens"], aux["denc"])
